# revision 33
# baseline (speedup 1.0000x reference)
"""Bahdanau attention via a rank-5 product-of-tanh-powers expansion.

score[t,s] = sum_u V_u tanh(a_su + d_tu)  (a = enc@W1+b1, d = dec@W2+b2)
is approximated by
  sum_{j=1..5} ta^j * g_j(td),  ta = tanh(ALPHA*a), td = tanh(BETA*d),
  g_j(td) = sum_m C[j][m] td^m  (parity: c_jm = 0 unless j+m odd; the
  j=0 block of the fit is softmax-invariant and never computed).
Device-faithful fp16 numpy sim of this pipeline: e2e rel err 0.0054.

The score is computed TRANSPOSED ([s-part, t]) so the softmax exp feeds the
context matmul directly (enc-with-ones-column rhs gives the denominator) with
no PE transposes of the attention matrix.

All inputs arrive in 2 packed DMAs (params+encT fp16; enc-with-ones fp16);
one f32 output DMA. Two pipeline copies per For_i iteration (tile pools with
bufs=2) overlap each iteration's tail with the next one's front.

kernel(**inputs) takes full unsharded arrays, shards batch across 8 cores,
returns full [8, 64, 256] f32 output.
"""

import numpy as np

import concourse.bass as bass
import concourse.tile as tile
from concourse import bacc
from concourse import mybir
from concourse import bass_utils

B, TD, TE, D, U = 8, 64, 1024, 256, 128
P = 128
F32 = mybir.dt.float32
FP16 = mybir.dt.float16
AF = mybir.ActivationFunctionType
OP = mybir.AluOpType

# ---- offline fit (fit_m5.py): tanh(a+d) ~ sum_j ta^j g_j(td) ----
ALPHA, BETA, K = 0.6, 0.6, 5
C = {
    1: {0: 1.654330, 2: -4.163001, 4: 2.818521},
    2: {1: -4.174277, 3: 9.012424, 5: -4.308858},
    3: {0: -0.820326, 2: 8.927075, 4: -10.878940},
    4: {1: 2.836418, 3: -11.014633, 5: 7.581800},
    5: {0: 0.131692, 2: -4.201689, 4: 7.413761},
}
CV_ORDER = [(j, m) for j in range(1, K + 1) for m in sorted(C[j])]
NSM = 2 + len(CV_ORDER)  # alpha*b1, beta*b2, V-scaled coeff columns

# big1 fp16 column layout (dec is packed pre-transposed: [d-part, t])
W1C, W2C, DECC, SMC, ENCTC = 0, 256, 512, 640, 664
NB1 = ENCTC + 2 * TE   # 2712
NB2 = 8 * (D + 1)      # 2056


def _stage_a(tc: tile.TileContext, pools: dict, ins: dict):
    """DMAs, d path (wdec/td/powers/combos), wenc matmuls."""
    nc = tc.nc
    sb, psD = pools["sb"], pools["psD"]

    big1 = sb.tile([P, NB1], FP16, tag="big1")
    big2f = sb.tile([P, NB2], FP16, tag="big2")
    nc.sync.dma_start(big1, ins["big1"])
    nc.sync.dma_start(big2f, ins["big2"])
    big2 = big2f.rearrange("p (c d) -> p c d", c=8)

    # per-partition scalars to f32 (activation bias / tensor_scalar ptrs)
    sm = sb.tile([P, NSM], F32, tag="sm")
    nc.gpsimd.tensor_copy(sm, big1[:, SMC:SMC + NSM])
    ab1, bb2 = sm[:, 0:1], sm[:, 1:2]

    def cv(j, m):
        i = 2 + CV_ORDER.index((j, m))
        return sm[:, i:i + 1]

    # score psum allocated early: its first 64 cols double as w_dec scratch
    # (f32, naturally ordered: wdec group closes before the score groups open)
    score = psD.tile([P, 8, TD], F32, tag="score")

    # ---- d path (dec arrives pre-transposed in big1) ----
    wdec = score[:, 0]
    for k in range(2):
        nc.tensor.matmul(wdec, big1[:, W2C + 128 * k:W2C + 128 * (k + 1)],
                         big1[:, DECC + TD * k:DECC + TD * (k + 1)],
                         start=(k == 0), stop=(k == 1))

    # ---- a path matmuls (PE continues while ACT/DVE chew the d path) ----
    wenc = psD.tile([U, TE], F32, tag="wenc")
    for h in range(2):
        for k in range(2):
            nc.tensor.matmul(
                wenc[:, 512 * h:512 * (h + 1)],
                big1[:, W1C + 128 * k:W1C + 128 * (k + 1)],
                big1[:, ENCTC + TE * k + 512 * h:ENCTC + TE * k + 512 * (h + 1)],
                start=(k == 0), stop=(k == 1))

    # td powers: slot m-1 holds td^m
    tDp = sb.tile([U, 5, TD], FP16, tag="tDp")
    nc.scalar.activation(tDp[:, 0], wdec, AF.Tanh, bias=bb2, scale=BETA)
    nc.gpsimd.tensor_tensor(tDp[:, 1], tDp[:, 0], tDp[:, 0], OP.mult)
    nc.gpsimd.tensor_tensor(tDp[:, 2], tDp[:, 1], tDp[:, 0], OP.mult)
    nc.gpsimd.tensor_tensor(tDp[:, 3], tDp[:, 1], tDp[:, 1], OP.mult)
    nc.gpsimd.tensor_tensor(tDp[:, 4], tDp[:, 2], tDp[:, 1], OP.mult)

    # rhs_j[u,t] = V_u * g_j(td) via V-prescaled coefficient ptrs
    rhsd = sb.tile([U, K, TD], FP16, tag="rhsd")
    for j in range(1, K + 1):
        e = nc.vector
        ms = sorted(C[j])
        out = rhsd[:, j - 1]
        if ms[0] == 0:
            e.tensor_scalar(out, tDp[:, ms[1] - 1], cv(j, ms[1]), cv(j, 0),
                            OP.mult, OP.add)
            rest = ms[2:]
        else:
            e.tensor_scalar(out, tDp[:, ms[0] - 1], cv(j, ms[0]), None, OP.mult)
            rest = ms[1:]
        for m in rest:
            e.scalar_tensor_tensor(out, tDp[:, m - 1], cv(j, m), out,
                                   OP.mult, OP.add)

    return dict(big2=big2, sm=sm, score=score, wenc=wenc, rhsd=rhsd, ab1=ab1)


def _stage_c(tc: tile.TileContext, pools: dict, st: dict):
    """ta + powers, transposed score matmuls, exp."""
    nc = tc.nc
    sb = pools["sb"]
    score, wenc, rhsd, ab1 = st["score"], st["wenc"], st["rhsd"], st["ab1"]

    tA = sb.tile([U, K, TE], FP16, tag="tA")
    for h in range(2):
        sl = slice(512 * h, 512 * (h + 1))
        nc.scalar.activation(tA[:, 0, sl], wenc[:, sl], AF.Tanh,
                             bias=ab1, scale=ALPHA)
        if h == 0:
            nc.scalar.activation(tA[:, 1, sl], tA[:, 0, sl], AF.Square)
        else:
            nc.vector.tensor_tensor(tA[:, 1, sl], tA[:, 0, sl], tA[:, 0, sl], OP.mult)
        nc.vector.tensor_tensor(tA[:, 2, sl], tA[:, 1, sl], tA[:, 0, sl], OP.mult)
        nc.vector.tensor_tensor(tA[:, 3, sl], tA[:, 1, sl], tA[:, 1, sl], OP.mult)
        nc.vector.tensor_tensor(tA[:, 4, sl], tA[:, 2, sl], tA[:, 1, sl], OP.mult)

    for c in range(8):
        for j in range(1, K + 1):
            nc.tensor.matmul(score[:, c], tA[:, j - 1, 128 * c:128 * (c + 1)],
                             rhsd[:, j - 1], start=(j == 1), stop=(j == K))
    ET = sb.tile([P, 8, TD], FP16, tag="ET")
    for h in range(2):
        nc.scalar.activation(ET[:, 4 * h:4 * (h + 1)],
                             score[:, 4 * h:4 * (h + 1)], AF.Exp)
    st["ET"] = ET


def _stage_b_ctx(tc: tile.TileContext, pools: dict, st: dict):
    """context matmuls (PE), placed between wenc(i) and score(i)."""
    nc = tc.nc
    psD = pools["psD"]
    ET, big2 = st["ET"], st["big2"]
    ctx = psD.tile([TD, D + 1], F32, tag="ctx")
    for c in range(8):
        nc.tensor.matmul(ctx, ET[:, c], big2[:, c], start=(c == 0), stop=(c == 7))
    st["ctx"] = ctx


def _stage_b_out(tc: tile.TileContext, pools: dict, st: dict, outd):
    """unnormalized output: ACT copy (after exp(i)) + Pool SWDGE DMA."""
    nc = tc.nc
    sb = pools["sb"]
    out_sb = sb.tile([TD, D + 1], F32, tag="out_sb")
    nc.scalar.activation(out_sb, st["ctx"], AF.Copy)
    nc.gpsimd.dma_start(outd, out_sb)


_CACHE = {}


def _get_nc(reps=1):
    if ("nc", reps) in _CACHE:
        return _CACHE[("nc", reps)]
    nc = bacc.Bacc("TRN2", target_bir_lowering=False, debug=False,
                   enable_asserts=True, num_devices=B)
    ins = {
        "big1": nc.dram_tensor("big1", [P, NB1], FP16, kind="ExternalInput").ap(),
        "big2": nc.dram_tensor("big2", [P, NB2], FP16, kind="ExternalInput").ap(),
    }
    outd = nc.dram_tensor("out", [TD, D + 1], F32, kind="ExternalOutput").ap()
    from contextlib import ExitStack
    with tile.TileContext(nc) as tc:
        with ExitStack() as es:
            stat = es.enter_context(tc.tile_pool(name="stat", bufs=1))
            sb = es.enter_context(tc.tile_pool(name="sb", bufs=2))
            psD = es.enter_context(tc.tile_pool(name="psD", bufs=2, space="PSUM"))
            # warm the activation-LUT set before the loop so the table-load
            # fixpoint can keep InstLoadActFuncSet out of the loop body
            warm = stat.tile([1, 2], FP16, tag="warm")
            nc.gpsimd.memset(warm, 0.0)
            nc.scalar.activation(warm[:, 0:1], warm[:, 1:2], AF.Tanh)
            pools = dict(sb=sb, psD=psD)

            def body(ncop):
                prev = None
                for _ in range(ncop):
                    st = _stage_a(tc, pools, ins)
                    if prev is not None:
                        _stage_b_ctx(tc, pools, prev)
                    _stage_c(tc, pools, st)
                    if prev is not None:
                        _stage_b_out(tc, pools, prev, outd)
                    prev = st
                _stage_b_ctx(tc, pools, prev)
                _stage_b_out(tc, pools, prev, outd)

            if reps == 1:
                body(1)
            else:
                ncop = 6 if reps % 6 == 0 else 2
                assert reps % ncop == 0, "reps must be divisible by 2"
                with tc.For_i(0, reps // ncop, 1):
                    body(ncop)
    nc.compile()
    _CACHE[("nc", reps)] = nc
    return nc


def _in_maps(decoder_output, encoder_output, W1, b1, W2, b2, V):
    f32, f16 = np.float32, np.float16
    W1 = np.asarray(W1, f32)
    W2 = np.asarray(W2, f32)
    b1 = np.asarray(b1, f32)
    b2 = np.asarray(b2, f32)
    Vf = np.asarray(V, f32).reshape(U)
    # per-partition scalar columns (V pre-folded into the d-side coeffs)
    sm = np.zeros((P, NSM), f32)
    sm[:, 0] = ALPHA * b1
    sm[:, 1] = BETA * b2
    for i, (j, m) in enumerate(CV_ORDER):
        sm[:, 2 + i] = C[j][m] * Vf
    maps = []
    for b in range(B):
        enc = np.asarray(encoder_output[b], f32)
        dec = np.asarray(decoder_output[b], f32)
        big1 = np.zeros((P, NB1), f16)
        big1[:, W1C + 0:W1C + 128] = W1[0:128]
        big1[:, W1C + 128:W1C + 256] = W1[128:256]
        big1[:, W2C + 0:W2C + 128] = W2[0:128]
        big1[:, W2C + 128:W2C + 256] = W2[128:256]
        big1[:, DECC:DECC + 64] = dec[:, 0:128].T
        big1[:, DECC + 64:DECC + 128] = dec[:, 128:256].T
        big1[:, SMC:SMC + NSM] = sm
        encT = np.ascontiguousarray(enc.T)
        big1[:, ENCTC:ENCTC + TE] = encT[0:128]
        big1[:, ENCTC + TE:ENCTC + 2 * TE] = encT[128:256]
        big2 = np.empty((P, NB2), f16)
        for k in range(8):
            big2[:, k * (D + 1):k * (D + 1) + D] = enc[k * 128:(k + 1) * 128]
            big2[:, k * (D + 1) + D] = 1.0
        maps.append({"big1": big1, "big2": big2})
    return maps


def run(decoder_output, encoder_output, W1, b1, W2, b2, V, bV=None, *,
        trace=False, **trace_kwargs):
    nc = _get_nc()
    maps = _in_maps(decoder_output, encoder_output, W1, b1, W2, b2, V)
    res = bass_utils.run_bass_kernel_spmd(
        nc, maps, core_ids=list(range(B)), trace=trace, **trace_kwargs)
    raw = np.stack([r["out"] for r in res.results], axis=0)  # [B, TD, D+1]
    out = raw[:, :, :D] / raw[:, :, D:D + 1]
    return out.astype(np.float32), res


def kernel(decoder_output, encoder_output, W1, b1, W2, b2, V, bV=None):
    out, _ = run(decoder_output, encoder_output, W1, b1, W2, b2, V, bV)
    return out
